# revision 1
# baseline (speedup 1.0000x reference)
"""Distributed Trainium2 kernel for a single causal attention head.

Problem (hardcoded): B=4, S=2048, D_MODEL=1024, HEAD_DIM=64, fp32 inputs.
    q = query @ Wq + bq ; k = key @ Wk + bk ; v = value @ Wv + bv
    scores = q k^T / sqrt(H) ; masked softmax ; out = att @ v

Sharding (8 NeuronCores): core c = (b, h) with b = c//2, h = c%2.
Each core handles two 512-row query chunks of batch b.  To balance causal
work with a single SPMD program, chunk pairing is h=0 -> global chunks
{0, 3}, h=1 -> {1, 2}; the program has two "slots" with fixed k-extents
of 8 and 16 j-tiles (j-tile = 128 keys).  All per-core differences are
data: the gathered query rows and host-built predicate masks (taken from
the real `mask` input) that zero attention weights after exp.

Device layout trick: query/key/value shards are passed TRANSPOSED
([D, S] bf16) so every matmul contracts over the partition dim with
operands in their natural layout (no on-device input transposes):
  qT[h,i]   = Wq^T Xq^T   (lhsT=Wq chunk, rhs=XqT chunk)
  kT[h,j]   = Wk^T Xk^T
  v[j,h]    = Xv^T-chunk as lhsT, rhs=Wv chunk  -> natural [j, h]
  sT[j,i]   = kT-tile as lhsT, rhs=qT           (scores transposed)
  att       = exp(sT * 0.125)  (ScalarE, PSUM->SBUF, bf16)
  oT[65,i] += v_aug-tile as lhsT, rhs=att       (v_aug has a ones column,
                                                 row 64 = softmax denom)
Final: PE-transpose [65,128] blocks, multiply by reciprocal of column 64,
DMA out as [i, 64] fp32.
"""

import os

import numpy as np
import ml_dtypes

import concourse.bass as bass
import concourse.tile as tile
from concourse import bacc, mybir
from concourse.bass import ds
from concourse.bass_utils import run_bass_kernel_spmd
from concourse.masks import make_identity

B, S, D, H = 4, 2048, 1024, 64
P = 128
NCORES = 8
CHUNK = 512               # query rows per slot
NSLOTS = 2                # slots per core (2 x 512 = 1024 q rows/core)
JT = S // P               # 16 j-tiles of 128 keys
FP = mybir.dt.float32
BF = mybir.dt.bfloat16
U8 = mybir.dt.uint8
BF_NP = ml_dtypes.bfloat16

# causal variant: slot extents in j-tiles and chunk assignment per h
CAUSAL_EXTENTS = (8, 16)
CAUSAL_CHUNKS = {0: (0, 3), 1: (1, 2)}
# slots that get a predicate (union of needs across h): slot0 all 8,
# slot1 upper 8
CAUSAL_MASKED = [(0, jt) for jt in range(8)] + [(1, jt) for jt in range(8, 16)]

FULL_EXTENTS = (16, 16)
FULL_CHUNKS = {0: (0, 1), 1: (2, 3)}
FULL_MASKED = [(s, jt) for s in range(2) for jt in range(16)]

# module global: last BassKernelResults (test.py reads exec_time_ns)
LAST_RESULTS = None


def _build_program(extents, masked_slots):
    """Build the SPMD Bass program (same for all 8 cores)."""
    nc = bacc.Bacc("TRN2", target_bir_lowering=False, debug=False,
                   num_devices=NCORES)

    NQ = NSLOTS * CHUNK  # 1024 query rows per core
    qT_d = nc.dram_tensor("qT", [D, NQ], BF, kind="ExternalInput").ap()
    kT_d = nc.dram_tensor("kT", [D, S], BF, kind="ExternalInput").ap()
    vT_d = nc.dram_tensor("vT", [D, S], BF, kind="ExternalInput").ap()
    wq_d = nc.dram_tensor("wq", [D, H], BF, kind="ExternalInput").ap()
    wk_d = nc.dram_tensor("wk", [D, H], BF, kind="ExternalInput").ap()
    wv_d = nc.dram_tensor("wv", [D, H], BF, kind="ExternalInput").ap()
    bq_d = nc.dram_tensor("bq", [H, 1], FP, kind="ExternalInput").ap()
    bk_d = nc.dram_tensor("bk", [H, 1], FP, kind="ExternalInput").ap()
    bv_d = nc.dram_tensor("bv", [P, H], FP, kind="ExternalInput").ap()
    nmask = len(masked_slots)
    pred_d = nc.dram_tensor("pred", [nmask, P, CHUNK], U8,
                            kind="ExternalInput").ap()
    out_d = nc.dram_tensor("out", [NQ, H], FP, kind="ExternalOutput").ap()

    DCH = D // P  # 8 contraction chunks

    with tile.TileContext(nc) as tc:
        with (
            tc.tile_pool(name="const", bufs=1) as const,
            tc.tile_pool(name="resident", bufs=1) as res,
            tc.tile_pool(name="attp", bufs=4) as attp,
            tc.tile_pool(name="outp", bufs=2) as outp,
            tc.tile_pool(name="ppq", bufs=2, space="PSUM") as ppq,
            tc.tile_pool(name="ppv", bufs=2, space="PSUM") as ppv,
            tc.tile_pool(name="psc", bufs=2, space="PSUM") as psc,
            tc.tile_pool(name="pout", bufs=1, space="PSUM") as pout,
            tc.tile_pool(name="ptr", bufs=1, space="PSUM") as ptr,
        ):
            # ---- constants ----
            wq_sb = const.tile([P, DCH, H], BF, tag="wq")
            wk_sb = const.tile([P, DCH, H], BF, tag="wk")
            wv_sb = const.tile([P, DCH, H], BF, tag="wv")
            nc.sync.dma_start(wq_sb, wq_d.rearrange("(o p) h -> p o h", p=P))
            nc.sync.dma_start(wk_sb, wk_d.rearrange("(o p) h -> p o h", p=P))
            nc.sync.dma_start(wv_sb, wv_d.rearrange("(o p) h -> p o h", p=P))
            bq_sb = const.tile([H, 1], FP, tag="bq")
            bk_sb = const.tile([H, 1], FP, tag="bk")
            bv_sb = const.tile([P, H], FP, tag="bv")
            nc.sync.dma_start(bq_sb, bq_d)
            nc.sync.dma_start(bk_sb, bk_d)
            nc.sync.dma_start(bv_sb, bv_d)
            zeros_sb = const.tile([P, CHUNK], BF, tag="zeros")
            nc.vector.memset(zeros_sb, 0.0)
            ident = const.tile([P, P], FP, tag="ident")
            make_identity(nc, ident)

            # ---- resident inputs ----
            xq_sb = res.tile([P, DCH, NQ], BF, tag="xq")
            xk_sb = res.tile([P, DCH, S], BF, tag="xk")
            xv_sb = res.tile([P, DCH, S], BF, tag="xv")
            qT_r = qT_d.rearrange("(o p) i -> p o i", p=P)
            kT_r = kT_d.rearrange("(o p) s -> p o s", p=P)
            vT_r = vT_d.rearrange("(o p) s -> p o s", p=P)
            for o in range(DCH):
                nc.sync.dma_start(xq_sb[:, o, :], qT_r[:, o, :])
                nc.sync.dma_start(xk_sb[:, o, :], kT_r[:, o, :])
                nc.sync.dma_start(xv_sb[:, o, :], vT_r[:, o, :])
            pred_sb = res.tile([P, nmask, CHUNK], U8, tag="pred")
            nc.sync.dma_start(pred_sb, pred_d.rearrange("t p f -> p t f"))

            # ---- projections ----
            # q^T [64, NQ] and k^T [64, S], zero-padded to 128 partitions so
            # the attention matmuls contract over a full 128.
            q_sb = res.tile([P, NQ], BF, tag="q")
            k_sb = res.tile([P, S], BF, tag="k")
            nc.vector.memset(q_sb[H:, :], 0.0)
            nc.vector.memset(k_sb[H:, :], 0.0)
            for ic in range(NQ // CHUNK):
                pq = ppq.tile([H, CHUNK], FP, tag="pq")
                for d in range(DCH):
                    nc.tensor.matmul(pq, lhsT=wq_sb[:, d, :],
                                     rhs=xq_sb[:, d, ds(ic * CHUNK, CHUNK)],
                                     start=(d == 0), stop=(d == DCH - 1))
                nc.scalar.activation(q_sb[:H, ds(ic * CHUNK, CHUNK)], pq,
                                     mybir.ActivationFunctionType.Identity,
                                     bias=bq_sb)
            for ic in range(S // CHUNK):
                pk = ppq.tile([H, CHUNK], FP, tag="pq")
                for d in range(DCH):
                    nc.tensor.matmul(pk, lhsT=wk_sb[:, d, :],
                                     rhs=xk_sb[:, d, ds(ic * CHUNK, CHUNK)],
                                     start=(d == 0), stop=(d == DCH - 1))
                nc.scalar.activation(k_sb[:H, ds(ic * CHUNK, CHUNK)], pk,
                                     mybir.ActivationFunctionType.Identity,
                                     bias=bk_sb)
            # v natural [j, h] with a ones column at h=64 (softmax denom)
            v_sb = res.tile([P, JT, H + 1], BF, tag="v")
            for jt in range(JT):
                pv = ppv.tile([P, H], FP, tag="pv")
                for d in range(DCH):
                    nc.tensor.matmul(pv, lhsT=xv_sb[:, d, ds(jt * P, P)],
                                     rhs=wv_sb[:, d, :],
                                     start=(d == 0), stop=(d == DCH - 1))
                nc.vector.tensor_add(v_sb[:, jt, :H], pv, bv_sb)
                nc.vector.memset(v_sb[:, jt, H:], 1.0)

            # ---- attention ----
            mask_idx = {sj: i for i, sj in enumerate(masked_slots)}
            for s in range(NSLOTS):
                po = pout.tile([H + 1, CHUNK], FP, tag="po")
                ext = extents[s]
                for jt in range(ext):
                    ps = psc.tile([P, CHUNK], FP, tag="sc")
                    nc.tensor.matmul(ps, lhsT=k_sb[:, ds(jt * P, P)],
                                     rhs=q_sb[:, ds(s * CHUNK, CHUNK)],
                                     start=True, stop=True)
                    att = attp.tile([P, CHUNK], BF, tag="att")
                    nc.scalar.activation(att, ps,
                                         mybir.ActivationFunctionType.Exp,
                                         scale=0.125)
                    mi = mask_idx.get((s, jt))
                    if mi is not None:
                        nc.vector.copy_predicated(att, pred_sb[:, mi, :],
                                                  zeros_sb)
                    nc.tensor.matmul(po, lhsT=v_sb[:, jt, :], rhs=att,
                                     start=(jt == 0), stop=(jt == ext - 1))
                # epilogue: transpose + normalize + store
                oT_sb = outp.tile([P, CHUNK], FP, tag="oT")
                nc.vector.tensor_copy(oT_sb[:H + 1, :], po)
                for t in range(CHUNK // P):
                    pt = ptr.tile([P, P], FP, tag="tr")
                    nc.tensor.transpose(pt, oT_sb[:, ds(t * P, P)], ident)
                    recip = outp.tile([P, 1], FP, tag="recip")
                    nc.vector.reciprocal(recip, pt[:, H:H + 1])
                    o_sb = outp.tile([P, H], FP, tag="o")
                    nc.vector.tensor_scalar_mul(o_sb, pt[:, :H], recip)
                    nc.sync.dma_start(out_d[ds(s * CHUNK + t * P, P), :], o_sb)

    nc.compile()
    return nc


def _mask_fits_causal_variant(mask):
    """True if every unmasked (True) position lies inside the causal
    variant's computed region: chunk g (rows [512g, 512(g+1))) only
    attends keys j < extent(g)*128 where extent = 8 for chunks 0..1's
    slot... actually per-chunk bound: chunk 0 -> 1024, 1 -> 1024,
    2 -> 2048, 3 -> 2048 (slot extents by assignment below)."""
    # chunk -> extent (keys computed) per the slot it lands in:
    # h=0: chunk0 -> slot0 (8 jt = 1024), chunk3 -> slot1 (2048)
    # h=1: chunk1 -> slot0 (1024), chunk2 -> slot1 (2048)
    bounds = {0: 1024, 1: 1024, 2: 2048, 3: 2048}
    for g, bound in bounds.items():
        if bound < S:
            blk = mask[:, g * CHUNK:(g + 1) * CHUNK, bound:]
            if blk.any():
                return False
    return True


def kernel(query, key, value, mask, Wq, bq, Wk, bk, Wv, bv):
    global LAST_RESULTS
    query = np.asarray(query, dtype=np.float32)
    key = np.asarray(key, dtype=np.float32)
    value = np.asarray(value, dtype=np.float32)
    mask = np.asarray(mask).astype(bool)
    Wq = np.asarray(Wq, dtype=np.float32)
    Wk = np.asarray(Wk, dtype=np.float32)
    Wv = np.asarray(Wv, dtype=np.float32)
    bq = np.asarray(bq, dtype=np.float32)
    bk = np.asarray(bk, dtype=np.float32)
    bv = np.asarray(bv, dtype=np.float32)

    causal_ok = _mask_fits_causal_variant(mask)
    if causal_ok:
        extents, chunks_of, masked = CAUSAL_EXTENTS, CAUSAL_CHUNKS, CAUSAL_MASKED
    else:
        extents, chunks_of, masked = FULL_EXTENTS, FULL_CHUNKS, FULL_MASKED

    nc = _build_program(extents, masked)

    wq_bf = Wq.astype(BF_NP)
    wk_bf = Wk.astype(BF_NP)
    wv_bf = Wv.astype(BF_NP)
    bq_in = bq.reshape(H, 1)
    bk_in = bk.reshape(H, 1)
    bv_in = np.broadcast_to(bv.reshape(1, H), (P, H)).copy()

    in_maps = []
    for c in range(NCORES):
        b, h = divmod(c, 2)
        chunks = chunks_of[h]
        rows = [np.arange(g * CHUNK, (g + 1) * CHUNK) for g in chunks]
        q_rows = np.concatenate([query[b, r, :] for r in rows], axis=0)
        qT = np.ascontiguousarray(q_rows.T).astype(BF_NP)
        kT = np.ascontiguousarray(key[b].T).astype(BF_NP)
        vT = np.ascontiguousarray(value[b].T).astype(BF_NP)
        # predicate: 1 where attention weight must be zeroed
        pred = np.zeros((len(masked), P, CHUNK), dtype=np.uint8)
        for i, (s, jt) in enumerate(masked):
            blk = mask[b, chunks[s] * CHUNK:(chunks[s] + 1) * CHUNK,
                       jt * P:(jt + 1) * P]          # [i=512, j=128]
            pred[i] = (~blk.T).astype(np.uint8)       # [j=128, i=512]
        in_maps.append({
            "qT": qT, "kT": kT, "vT": vT,
            "wq": wq_bf, "wk": wk_bf, "wv": wv_bf,
            "bq": bq_in, "bk": bk_in, "bv": bv_in,
            "pred": pred,
        })

    results = run_bass_kernel_spmd(
        nc, in_maps, core_ids=list(range(NCORES)),
        trace=bool(os.environ.get("BASS_TRACE")),
    )
    LAST_RESULTS = results

    out = np.empty((B, S, H), dtype=np.float32)
    for c in range(NCORES):
        b, h = divmod(c, 2)
        chunks = chunks_of[h]
        o = results.results[c]["out"]
        for s, g in enumerate(chunks):
            out[b, g * CHUNK:(g + 1) * CHUNK, :] = o[s * CHUNK:(s + 1) * CHUNK]
    return out
